# revision 41
# baseline (speedup 1.0000x reference)
"""GroupedQueryAttention Trainium2 kernel (8 NeuronCores, SPMD), v2.

Sharding: 16 (batch, q-head) pairs over 8 cores -> core c handles batch c//4,
kv-head j=c%4, q-heads {2j, 2j+1}. Each core computes its heads' causal flash
attention plus its partial output projection; host sums the 4 partials per
batch.

Device-side layout is fully "transposed" (head_dim on partitions, sequence on
free dim):
  scores^T[k, q] = kT_chunk.T @ qT     (row-banded tile_position matmuls, K=32;
                                        the 4 bands stream concurrently)
  P^T = exp(scores^T * 1/sqrt(hd))     (ACT exp, or 1-op DVE Schraudolph
                                        fast-exp writing bf16 bits as int16)
  out^T[hd, q] = v_aug.T @ P^T         (v_aug has a leading ones column -> row
                                        0 of the accumulator is the softmax
                                        denominator; the two q-heads' PV
                                        matmuls are column-tiled to array cols
                                        0-63 / 64-127 and run concurrently)

Differences vs v1 (276-310us):
 - ascending-block schedule: block qb's attention needs only prologues 0..qb,
   so compute starts ~4us in instead of ~90us (DMAs are chunked per block).
 - qT/kT band replication is done by the PE (repmat matmul) + one DVE cast,
   not 12 SBUF->SBUF DMAs per block (those serialized ~60us on the sync queue).
 - causal mask applied by an accumulating matmul (maskT @ I) on the diagonal
   chunks, not DVE multiplies.
 - softmax reciprocal via the 1-op custom-DVE reciprocal_approx_fast on a
   [65,512] tile covering both heads (v1: 2x 3.3us hardware reciprocal).
 - out-projection row-tiled: gt0 at array rows 0-32, gt1 at rows 64-96,
   accumulating into one PSUM bank.
 - a fraction of exp tiles runs on the DVE as a single tensor_scalar
   (Schraudolph fast-exp into bf16 bits); rel err stays < 1e-2 because
   attention here is diffuse (scaled scores span ~[-1,1]).
"""

import json
import sys

import numpy as np

for _p in ("/opt/trn_rl_repo",):
    if _p not in sys.path:
        try:
            import concourse.bass  # noqa: F401
        except Exception:
            sys.path.insert(0, _p)
    break

import concourse.bass as bass
import concourse.tile as tile
from concourse import mybir
from concourse.bass_utils import run_bass_kernel_spmd

F32 = mybir.dt.float32
BF16 = mybir.dt.bfloat16
I16 = mybir.dt.int16
NP_BF16 = mybir.dt.np(BF16)

B, S, H = 2, 4096, 256
NH, NKV, HD = 8, 4, 32
QB = 512                   # q block width
NQB = S // QB              # 8
KC = 128                   # k chunk
SCALE = 1.0 / np.sqrt(HD)
ROPE_BASE = 10000.0
MV = 300.0                 # causal mask subtractor on raw scores
LOG2E = float(np.log2(np.e))
# Schraudolph int16/bf16 fast-exp: bits = round(y*log2e*128 + (127*128 - C16))
C16 = 5.25
SCH_MUL = float(SCALE * LOG2E * 128.0)
SCH_ADD = float(127.0 * 128.0 - C16)

# which (g,h) units compute exp on DVE instead of ACT: every unit with
# (unit_idx % OFFLOAD_MOD) < OFFLOAD_CNT is offloaded.
OFFLOAD_MOD = 3
OFFLOAD_CNT = 1
# zero-adding filler matmuls per group: keep the PE instruction stream
# gap-free so the HAM clock gate stays at 2.4GHz (cold PE runs 2x slower;
# measured 190us of the 263us v2a span at k=4/8 without fillers). Early
# blocks have more PE slack (prologue/DMA paced), late blocks are nearly
# PE-saturated with real work, so the filler count tapers off.
FILL_BY_BLOCK = [4, 4, 4, 3, 3, 3, 3, 3]


# ---------------------------------------------------------------- wait fixup
def _fix_waits_json(bir_bytes: bytes) -> bytes:
    """walrus (gen3) allows only one sync-wait per instruction struct; hoist
    extra waits onto inserted same-engine NoOps."""
    m = json.loads(bir_bytes)
    counter = 0
    for f in m.get("functions", []):
        for blk in f.get("blocks", []):
            out = []
            for inst in blk.get("instructions", []):
                si = inst.get("sync_info") or {}
                waits = si.get("on_wait") or []
                keep = 0 if inst.get("opcode") == "Matmult" else 1
                if len(waits) > keep:
                    for wsub in waits[keep:]:
                        counter += 1
                        out.append({
                            "debug": inst.get("debug", 0),
                            "engine": inst["engine"],
                            "ins": [],
                            "outs": [],
                            "name": f"waitfix-{counter}",
                            "opcode": "NoOp",
                            "sync_info": {"on_update": [], "on_wait": [wsub]},
                        })
                    si["on_wait"] = waits[:keep]
                out.append(inst)
            blk["instructions"] = out
    return json.dumps(m).encode()


def _install_waitfix(nc):
    orig = nc.to_json_bytes

    def patched(*a, **k):
        return _fix_waits_json(orig(*a, **k))

    nc.to_json_bytes = patched


# ---------------------------------------------------------------- device code
def _build_module():
    nc = bass.Bass()

    hsT = nc.declare_dram_parameter("hsT", [128, 2, S], BF16, isOutput=False)
    wpack = nc.declare_dram_parameter("wpack", [128, 2, 224], BF16, isOutput=False)
    cspack = nc.declare_dram_parameter("cspack", [96, NQB, 2 * QB], BF16, isOutput=False)
    gtpack = nc.declare_dram_parameter("gtpack", [97, 2, 128], BF16, isOutput=False)
    mpack = nc.declare_dram_parameter("mpack", [128, 384], BF16, isOutput=False)
    out_part = nc.declare_dram_parameter("out_part", [128, 2, S], F32, isOutput=True)

    with tile.TileContext(nc) as tc:
        with (
            tc.tile_pool(name="const", bufs=1) as const,
            tc.tile_pool(name="qtp", bufs=4) as qtp,
            tc.tile_pool(name="qkp", bufs=2) as qkp,
            tc.tile_pool(name="ptp", bufs=8) as ptp,
            tc.tile_pool(name="up", bufs=2) as up,
            tc.tile_pool(name="outp", bufs=2) as outp,
            tc.tile_pool(name="ps_sc", bufs=2, space="PSUM") as ps_sc,
            tc.tile_pool(name="ps_pv", bufs=1, space="PSUM") as ps_pv,
            tc.tile_pool(name="ps_mm", bufs=3, space="PSUM") as ps_mm,
        ):
            # ---- persistent tiles
            hsT_sb = const.tile([128, 2, S], BF16)
            kT_rep = const.tile([128, S], BF16)
            # per chunk: col 0 = ones (softmax denominator row), cols 1-32 =
            # v, cols 33-63 = zeros (pad so PV writes a full 64-row band)
            v_all = const.tile([128, S // KC, 64], BF16)
            cs_sb = const.tile([96, NQB, 2 * QB], BF16)
            wpack_sb = const.tile([128, 2, 224], BF16)
            gt_sb = const.tile([97, 2, 128], BF16)
            mpack_sb = const.tile([128, 384], BF16)
            ones_t = const.tile([65, 64], BF16)
            eps_t = const.tile([65, 1], F32)
            dumm = const.tile([1, 8], BF16)

            tri_sb = mpack_sb[:, 0:128]
            repmat = mpack_sb[0:96, 256:384]

            # ---- PE warm-up while the first DMAs fly
            wtmp = const.tile([128, QB], BF16)
            nc.vector.memset(wtmp[:], 0.0)
            nc.vector.memset(ones_t[:], 1.0)
            nc.vector.memset(v_all[:, :, 0:1], 1.0)
            nc.vector.memset(v_all[:, :, 33:64], 0.0)
            nc.vector.memset(eps_t[:], 1e-6)
            nc.vector.memset(dumm[:], 0.0)
            for w in range(8):
                pwarm = ps_mm.tile([128, QB], F32, tag="mm", name="warm")
                nc.tensor.matmul(pwarm[:], wtmp[:, 0:128], wtmp[:],
                                 start=True, stop=True)
            # pre-trigger the ACT exp table load with a dummy op
            dume = const.tile([1, 8], BF16)
            nc.scalar.activation(out=dume[:], in_=dumm[:],
                                 func=mybir.ActivationFunctionType.Exp)

            # ---- DMAs, chunked per block, in the order blocks consume them
            nc.sync.dma_start(out=wpack_sb[:], in_=wpack[:])

            def dma_block_inputs(qb):
                sl = slice(QB * qb, QB * (qb + 1))
                nc.sync.dma_start(out=hsT_sb[:, :, sl], in_=hsT[:, :, sl])
                nc.sync.dma_start(out=cs_sb[:, qb, :], in_=cspack[:, qb, :])

            dma_block_inputs(0)
            nc.sync.dma_start(out=mpack_sb[:], in_=mpack[:])
            dma_block_inputs(1)
            nc.sync.dma_start(out=gt_sb[:], in_=gtpack[:])
            for qb in range(2, NQB):
                dma_block_inputs(qb)

            qt_state = {}

            live_pv = [None]

            def emit_prologue(qb):
                """q/k/v projections + RoPE + PE band replication for block qb.
                Zero-add fillers into the last actively-written PV accumulator
                keep the PE busy across the DVE-gated RoPE/cast chain."""
                def fill(n):
                    if live_pv[0] is None:
                        return
                    for _ in range(n):
                        nc.tensor.matmul(live_pv[0][0:64, :], wtmp[:, 0:64],
                                         wtmp[:], start=False, stop=False,
                                         skip_group_check=True)
                sl = slice(QB * qb, QB * (qb + 1))
                p_qk = ps_mm.tile([96, QB], F32, tag="mm", name="p_qk")
                p_qkr = ps_mm.tile([96, QB], F32, tag="mm", name="p_qkr")
                for c in range(2):
                    nc.tensor.matmul(p_qk[:], wpack_sb[:, c, 0:96],
                                     hsT_sb[:, c, sl],
                                     start=(c == 0), stop=(c == 1))
                for c in range(2):
                    nc.tensor.matmul(p_qkr[:], wpack_sb[:, c, 96:192],
                                     hsT_sb[:, c, sl],
                                     start=(c == 0), stop=(c == 1))
                t1 = qkp.tile([96, QB], BF16, tag="t1", name="t1")
                t2 = qkp.tile([96, QB], BF16, tag="t2", name="t2")
                qkT = qkp.tile([96, QB], BF16, tag="qkT", name="qkT")
                nc.vector.tensor_mul(t1[:], p_qk[:], cs_sb[:, qb, 0:QB])
                nc.vector.tensor_mul(t2[:], p_qkr[:], cs_sb[:, qb, QB:2 * QB])
                nc.vector.tensor_add(qkT[:], t1[:], t2[:])
                fill(2)

                # replicate q0/q1/k across the 4 row bands with the PE
                qt_h = [qtp.tile([128, QB], BF16, tag=f"qt{h}", name=f"qt{h}")
                        for h in range(2)]
                reps = []
                for a in range(3):
                    rp = ps_mm.tile([128, QB], F32, tag="mm", name=f"rep{a}")
                    nc.tensor.matmul(rp[:], repmat[32 * a:32 * (a + 1), :],
                                     qkT[32 * a:32 * (a + 1), :],
                                     start=True, stop=True,
                                     tile_position=(32 * a, 0),
                                     skip_group_check=True)
                    reps.append(rp)
                with nc.allow_low_precision(reason="bf16 q/k bands"):
                    nc.vector.tensor_copy(qt_h[0][:], reps[0][:])
                    nc.vector.tensor_copy(qt_h[1][:], reps[1][:])
                    nc.vector.tensor_copy(kT_rep[:, sl], reps[2][:])
                fill(2)

                # v projection for this block's 4 s-chunks
                vp = ps_mm.tile([128, 4, HD], F32, tag="mm", name="vproj")
                for s4 in range(4):
                    sblk = 4 * qb + s4
                    for c in range(2):
                        nc.tensor.matmul(
                            vp[:, s4, :],
                            hsT_sb[:, c, KC * sblk:KC * (sblk + 1)],
                            wpack_sb[:, c, 192:224],
                            start=(c == 0), stop=(c == 1),
                            skip_group_check=True)
                with nc.allow_low_precision(reason="bf16 v"):
                    nc.vector.tensor_copy(
                        v_all[:, 4 * qb:4 * qb + 4, 1:HD + 1], vp[:])
                qt_state[qb] = qt_h

            unit_idx = [0]

            def emit_scores_exp(qb, g, h, qt_h):
                """scores + exp for one (block, group, head); the causal mask
                on the two diagonal 128x128 sub-blocks is a DVE multiply by a
                lower-triangular table on the exp output."""
                diag0 = (g == 2 * qb)
                diag1 = (g == 2 * qb + 1)
                qlo = [0, 0]
                if diag0:
                    qlo = [0, KC]
                elif diag1:
                    qlo = [2 * KC, 3 * KC]
                sc = ps_sc.tile([128, 2 * QB], F32, tag="sc", name="sc")
                # for diag0 the exp is one contiguous op over [0:1024], so the
                # d=1 chunk writes from column QB (no uninitialized hole); for
                # diag1 the exp is split per chunk and both trims stay.
                qmm = [qlo[0], 0 if not diag1 else qlo[1]]
                for d in range(2):
                    ch = 2 * g + d
                    t = ch % 4
                    nc.tensor.matmul(
                        sc[:, QB * d + qmm[d]:QB * (d + 1)],
                        kT_rep[32 * t:32 * (t + 1), KC * ch:KC * (ch + 1)],
                        qt_h[h][32 * t:32 * (t + 1), qmm[d]:],
                        start=True, stop=True,
                        tile_position=(32 * t, 0),
                        skip_group_check=True)
                pt = ptp.tile([128, 2 * QB], BF16, tag="pt", name="pt")
                ui = unit_idx[0]
                unit_idx[0] += 1
                offl = (ui % OFFLOAD_MOD) < OFFLOAD_CNT
                regions = ([(qlo[0], 2 * QB)] if not diag1 else
                           [(qlo[0], QB), (QB + qlo[1], 2 * QB)])
                for (r0, r1) in regions:
                    if offl:
                        # 1-op DVE Schraudolph fast-exp: int16 bf16-bits
                        with nc.allow_low_precision(reason="schraudolph exp"):
                            nc.vector.tensor_scalar(
                                pt[:, r0:r1].bitcast(I16), sc[:, r0:r1],
                                SCH_MUL, SCH_ADD,
                                mybir.AluOpType.mult, mybir.AluOpType.add)
                    else:
                        nc.scalar.activation(
                            out=pt[:, r0:r1], in_=sc[:, r0:r1],
                            func=mybir.ActivationFunctionType.Exp,
                            scale=float(SCALE))
                if diag0 or diag1:
                    for d in range(2):
                        dg = 2 * g + d - 4 * qb
                        w0 = QB * d + KC * dg
                        nc.vector.tensor_mul(pt[:, w0:w0 + KC],
                                             pt[:, w0:w0 + KC], tri_sb)
                return pt

            def emit_pv(entry):
                """col-tiled PV pair: h0 -> pvacc[0:64], h1 -> pvacc[64:128]."""
                qb_, g_, pts, pvacc_ = entry
                first = (g_ == 0)
                last = (g_ == 2 * qb_ + 1)
                if first:
                    live_pv[0] = pvacc_
                if not first:
                    # zero-add fillers (N=256 quantum): K=128 so their row
                    # groups overlap everything; the PE runs them back-to-back
                    for _ in range(FILL_BY_BLOCK[qb_]):
                        nc.tensor.matmul(pvacc_[0:64, :], wtmp[:, 0:64],
                                         wtmp[:], start=False, stop=False,
                                         skip_group_check=True)
                for d in range(2):
                    ch = 2 * g_ + d
                    dg = ch - 4 * qb_
                    diag_ = (g_ >= 2 * qb_)
                    cols = KC * dg if diag_ else 0
                    for h in range(2):
                        # start marks pending-zero for this matmul's own
                        # partitions only, so both heads' first PV start=True
                        nc.tensor.matmul(
                            pvacc_[64 * h:64 * h + 64, cols:],
                            v_all[:, ch, :],
                            pts[h][:, QB * d + cols:QB * (d + 1)],
                            start=(first and d == 0),
                            stop=(last and d == 1),
                            tile_position=(0, 64 * h),
                            skip_group_check=True)

            norm_state = {}

            def emit_norm_sq(qb):
                """stage 1 of the softmax reciprocal 1/d = exp(-0.5*ln(d^2)):
                the squaring keeps the ln input positive on the (unused) rows
                holding unnormalized outputs, so no NaNs; +1e-6 keeps the
                zero-pad rows' ln finite. Must be emitted after block qb's
                last PV and before block qb+1's first PV (pvacc is
                single-buffered), as must the u cast."""
                pvacc = pv_state.pop(qb)
                sq = up.tile([65, QB], F32, tag="sq", name="sq")
                nc.scalar.activation(out=sq[:], in_=pvacc[0:65, :],
                                     func=mybir.ActivationFunctionType.Square,
                                     bias=eps_t[:])
                u_bf = up.tile([97, QB], BF16, tag="u", name="u")
                with nc.allow_low_precision(reason="unnorm attn bf16"):
                    nc.vector.tensor_copy(u_bf[:], pvacc[0:97, :])
                norm_state[qb] = (u_bf, sq)

            def emit_norm_ln(qb):
                u_bf, sq = norm_state[qb]
                lg = up.tile([65, QB], F32, tag="lg", name="lg")
                nc.scalar.activation(out=lg[:], in_=sq[:],
                                     func=mybir.ActivationFunctionType.Ln)
                norm_state[qb] = (u_bf, lg)

            def emit_norm_rc(qb):
                u_bf, lg = norm_state[qb]
                rc_bf = up.tile([65, QB], BF16, tag="rcb", name="rcb")
                with nc.allow_low_precision(reason="bf16 softmax denom"):
                    nc.scalar.activation(out=rc_bf[:], in_=lg[:],
                                         func=mybir.ActivationFunctionType.Exp,
                                         scale=-0.5)
                norm_state[qb] = (u_bf, rc_bf)

            def fill_live(n):
                if live_pv[0] is None:
                    return
                for _ in range(n):
                    nc.tensor.matmul(live_pv[0][0:64, :], wtmp[:, 0:64],
                                     wtmp[:], start=False, stop=False,
                                     skip_group_check=True)

            def emit_norm_mid(qb):
                """PE broadcast of the reciprocal across the 33 output rows."""
                u_bf, rc_bf = norm_state[qb]
                fill_live(2)
                bc_ps = ps_mm.tile([128, QB], F32, tag="mm", name="bc")
                for h in range(2):
                    nc.tensor.matmul(bc_ps[64 * h:64 * h + 64, :],
                                     ones_t[64 * h:64 * h + 1, :],
                                     rc_bf[64 * h:64 * h + 1, :],
                                     start=True, stop=True,
                                     tile_position=(64 * h, 64 * h),
                                     skip_group_check=True)
                norm_state[qb] = (u_bf, rc_bf, bc_ps)

            def emit_norm_post(qb):
                """normalize + output-project + store block qb."""
                u_bf, rc_bf, bc_ps = norm_state.pop(qb)
                fill_live(2)
                slp = slice(QB * qb, QB * (qb + 1))
                nT = up.tile([97, QB], BF16, tag="nT", name="nT")
                with nc.allow_low_precision(reason="softmax norm bf16"):
                    nc.vector.tensor_mul(nT[:], u_bf[:], bc_ps[0:97, :])
                po_sb = outp.tile([128, 2, QB], F32, tag="po", name="po")
                for m in range(2):
                    po = ps_mm.tile([128, QB], F32, tag="mm", name="outproj")
                    # K=65 slice (rows 33-64 are zeros) makes the two MMs'
                    # row groups overlap at strip 2, so the HW serializes
                    # them -- concurrent row-tiled MMs accumulating into the
                    # same PSUM elements race and fault.
                    nc.tensor.matmul(po[:], gt_sb[0:65, m, :], nT[0:65, :],
                                     start=True, stop=False,
                                     skip_group_check=True)
                    nc.tensor.matmul(po[:], gt_sb[64:97, m, :], nT[64:97, :],
                                     start=False, stop=True,
                                     tile_position=(64, 0),
                                     skip_group_check=True)
                    nc.vector.tensor_copy(po_sb[:, m, :], po[:])
                    for hh in range(2):
                        cs = slice(QB * qb + 256 * hh, QB * qb + 256 * (hh + 1))
                        nc.sync.dma_start(out=out_part[:, m, cs],
                                          in_=po_sb[:, m, 256 * hh:256 * (hh + 1)])

            # ---- main pipeline: ascending blocks.
            # PV runs 2 groups behind its scores/exp (defer queue), crossing
            # block boundaries: at block qb, the g=0 and g=1 pops emit the
            # previous block's last two PV pairs, so norm_pre(qb-1) goes right
            # after the g=1 pop and the g=2 pop emits this block's first PV.
            emit_prologue(0)
            emit_prologue(1)
            emit_prologue(2)
            pv_state = {}
            defer = []
            next_plg = [3]
            for qb in range(NQB):
                qt_h = qt_state.pop(qb)
                pvacc = ps_pv.tile([128, QB], F32, tag="pvacc", name="pvacc")
                pv_state[qb] = pvacc
                ngroups = 2 * (qb + 1)
                for g in range(ngroups):
                    pt0 = emit_scores_exp(qb, g, 0, qt_h)
                    pt1 = emit_scores_exp(qb, g, 1, qt_h)
                    defer.append((qb, g, [pt0, pt1], pvacc))
                    if len(defer) > 2:
                        emit_pv(defer.pop(0))
                    if qb > 0:
                        if ngroups >= 6:
                            if g == 1:
                                emit_norm_sq(qb - 1)
                            elif g == 2:
                                emit_norm_ln(qb - 1)
                            elif g == 3:
                                emit_norm_rc(qb - 1)
                            elif g == 4:
                                emit_norm_mid(qb - 1)
                            elif g == 5:
                                emit_norm_post(qb - 1)
                        else:
                            if g == 1:
                                emit_norm_sq(qb - 1)
                                emit_norm_ln(qb - 1)
                            elif g == 2:
                                emit_norm_rc(qb - 1)
                                emit_norm_mid(qb - 1)
                            elif g == 3:
                                emit_norm_post(qb - 1)
                    if next_plg[0] < NQB and next_plg[0] - qb <= 3 \
                            and g == min(1 if qb < 3 else 4, ngroups - 1):
                        emit_prologue(next_plg[0])
                        next_plg[0] += 1

            # ---- drain + final norm
            for entry in defer:
                emit_pv(entry)
            emit_norm_sq(NQB - 1)
            emit_norm_ln(NQB - 1)
            emit_norm_rc(NQB - 1)
            emit_norm_mid(NQB - 1)
            emit_norm_post(NQB - 1)

    _install_waitfix(nc)
    return nc


_NC_CACHE = {}


def _get_nc():
    if "nc" not in _NC_CACHE:
        _NC_CACHE["nc"] = _build_module()
    return _NC_CACHE["nc"]


# ---------------------------------------------------------------- host side
def _rope_tables():
    inv = 1.0 / (ROPE_BASE ** (np.arange(0, HD, 2, dtype=np.float64) / HD))
    t = np.arange(S, dtype=np.float64)
    freqs = np.outer(t, inv)                     # [S, 16]
    emb = np.concatenate([freqs, freqs], axis=1)  # [S, 32]
    cosT = np.cos(emb).T.astype(np.float32)      # [32, S]
    sinT = np.sin(emb).T.astype(np.float32)
    return np.tile(cosT, (3, 1)), np.tile(sinT, (3, 1))   # [96, S]


def _rot_rows(w):
    # rows of (rotate_half o) projection: row d<16 -> -w[d+16]; d>=16 -> w[d-16]
    return np.concatenate([-w[16:32], w[0:16]], axis=0)


def kernel(hidden_states, Wq, Wk, Wv, Wo):
    hidden_states = np.asarray(hidden_states, dtype=np.float32)
    Wq = np.asarray(Wq, dtype=np.float32)
    Wk = np.asarray(Wk, dtype=np.float32)
    Wv = np.asarray(Wv, dtype=np.float32)
    Wo = np.asarray(Wo, dtype=np.float32)

    cosT, sinT = _rope_tables()   # [96, S] each
    cspack = np.empty((96, NQB, 2 * QB), np.float32)
    for qb in range(NQB):
        sl = slice(QB * qb, QB * (qb + 1))
        cspack[:, qb, 0:QB] = cosT[:, sl]
        cspack[:, qb, QB:] = sinT[:, sl]

    # mpack: [tri (lower-triangular causal keep-mask) | unused | repmat3]
    mpack = np.zeros((128, 384), np.float32)
    mpack[:, 0:128] = (np.arange(KC)[:, None] <= np.arange(KC)[None, :])
    for a in range(3):
        for t in range(4):
            mpack[32 * a:32 * (a + 1), 256 + 32 * t:256 + 32 * (t + 1)] += \
                np.eye(32, dtype=np.float32)

    hsT_b = []
    for b in range(B):
        ht = np.ascontiguousarray(hidden_states[b].T)   # [256, S]
        hsT_b.append(np.ascontiguousarray(
            ht.reshape(2, 128, S).transpose(1, 0, 2)).astype(NP_BF16))

    in_maps = []
    for core in range(8):
        b, j = core // 4, core % 4
        Wq_h = Wq[64 * j:64 * j + 64]            # [64, 256]
        Wk_j = Wk[32 * j:32 * j + 32]            # [32, 256]
        Wqk = np.concatenate([Wq_h, Wk_j], axis=0)           # [96, 256]
        Wqkrot = np.concatenate(
            [_rot_rows(Wq_h[0:32]), _rot_rows(Wq_h[32:64]), _rot_rows(Wk_j)],
            axis=0)                                          # [96, 256]
        Wv_j = Wv[32 * j:32 * j + 32]            # [32, 256]
        # wpack[p, c, :] = [Wqk[:,128c+p] | Wqkrot[:,128c+p] | Wv[:,128c+p]]
        wpack = np.empty((128, 2, 224), np.float32)
        for c in range(2):
            cols = slice(128 * c, 128 * (c + 1))
            wpack[:, c, 0:96] = Wqk[:, cols].T
            wpack[:, c, 96:192] = Wqkrot[:, cols].T
            wpack[:, c, 192:224] = Wv_j[:, cols].T

        G = Wo[:, 64 * j:64 * j + 64]            # [256, 64]
        gtpack = np.zeros((97, 2, 128), np.float32)
        for m in range(2):
            rows = slice(128 * m, 128 * (m + 1))
            gtpack[1:33, m, :] = G[rows, 0:32].T
            gtpack[65:97, m, :] = G[rows, 32:64].T

        in_maps.append({
            "hsT": hsT_b[b],
            "wpack": wpack.astype(NP_BF16),
            "cspack": cspack.astype(NP_BF16),
            "gtpack": gtpack.astype(NP_BF16),
            "mpack": mpack.astype(NP_BF16),
        })

    nc = _get_nc()
    res = run_bass_kernel_spmd(nc, in_maps, list(range(8)), trace=False)

    out = np.empty((B, S, H), np.float32)
    for b in range(B):
        acc = np.zeros((H, S), np.float32)
        for j in range(4):
            r = np.asarray(res.results[4 * b + j]["out_part"], dtype=np.float32)
            acc += r.transpose(1, 0, 2).reshape(H, S)
        out[b] = acc.T
    return out


# revision 42
# speedup vs baseline: 1.0228x; 1.0228x over previous
"""GroupedQueryAttention Trainium2 kernel (8 NeuronCores, SPMD), v2.

Sharding: 16 (batch, q-head) pairs over 8 cores -> core c handles batch c//4,
kv-head j=c%4, q-heads {2j, 2j+1}. Each core computes its heads' causal flash
attention plus its partial output projection; host sums the 4 partials per
batch.

Device-side layout is fully "transposed" (head_dim on partitions, sequence on
free dim):
  scores^T[k, q] = kT_chunk.T @ qT     (row-banded tile_position matmuls, K=32;
                                        the 4 bands stream concurrently)
  P^T = exp(scores^T * 1/sqrt(hd))     (ACT exp, or 1-op DVE Schraudolph
                                        fast-exp writing bf16 bits as int16)
  out^T[hd, q] = v_aug.T @ P^T         (v_aug has a leading ones column -> row
                                        0 of the accumulator is the softmax
                                        denominator; the two q-heads' PV
                                        matmuls are column-tiled to array cols
                                        0-63 / 64-127 and run concurrently)

Differences vs v1 (276-310us):
 - ascending-block schedule: block qb's attention needs only prologues 0..qb,
   so compute starts ~4us in instead of ~90us (DMAs are chunked per block).
 - qT/kT band replication is done by the PE (repmat matmul) + one DVE cast,
   not 12 SBUF->SBUF DMAs per block (those serialized ~60us on the sync queue).
 - causal mask applied by an accumulating matmul (maskT @ I) on the diagonal
   chunks, not DVE multiplies.
 - softmax reciprocal via the 1-op custom-DVE reciprocal_approx_fast on a
   [65,512] tile covering both heads (v1: 2x 3.3us hardware reciprocal).
 - out-projection row-tiled: gt0 at array rows 0-32, gt1 at rows 64-96,
   accumulating into one PSUM bank.
 - a fraction of exp tiles runs on the DVE as a single tensor_scalar
   (Schraudolph fast-exp into bf16 bits); rel err stays < 1e-2 because
   attention here is diffuse (scaled scores span ~[-1,1]).
"""

import json
import sys

import numpy as np

for _p in ("/opt/trn_rl_repo",):
    if _p not in sys.path:
        try:
            import concourse.bass  # noqa: F401
        except Exception:
            sys.path.insert(0, _p)
    break

import concourse.bass as bass
import concourse.tile as tile
from concourse import mybir
from concourse.bass_utils import run_bass_kernel_spmd

F32 = mybir.dt.float32
BF16 = mybir.dt.bfloat16
I16 = mybir.dt.int16
NP_BF16 = mybir.dt.np(BF16)

B, S, H = 2, 4096, 256
NH, NKV, HD = 8, 4, 32
QB = 512                   # q block width
NQB = S // QB              # 8
KC = 128                   # k chunk
SCALE = 1.0 / np.sqrt(HD)
ROPE_BASE = 10000.0
MV = 300.0                 # causal mask subtractor on raw scores
LOG2E = float(np.log2(np.e))
# Schraudolph int16/bf16 fast-exp: bits = round(y*log2e*128 + (127*128 - C16))
C16 = 5.25
SCH_MUL = float(SCALE * LOG2E * 128.0)
SCH_ADD = float(127.0 * 128.0 - C16)

# which (g,h) units compute exp on DVE instead of ACT: every unit with
# (unit_idx % OFFLOAD_MOD) < OFFLOAD_CNT is offloaded.
OFFLOAD_MOD = 3
OFFLOAD_CNT = 1
# zero-adding filler matmuls per group: keep the PE instruction stream
# gap-free so the HAM clock gate stays at 2.4GHz (cold PE runs 2x slower;
# measured 190us of the 263us v2a span at k=4/8 without fillers). Early
# blocks have more PE slack (prologue/DMA paced), late blocks are nearly
# PE-saturated with real work, so the filler count tapers off.
FILL_BY_BLOCK = [4, 4, 4, 3, 2, 2, 2, 2]


# ---------------------------------------------------------------- wait fixup
def _fix_waits_json(bir_bytes: bytes) -> bytes:
    """walrus (gen3) allows only one sync-wait per instruction struct; hoist
    extra waits onto inserted same-engine NoOps."""
    m = json.loads(bir_bytes)
    counter = 0
    for f in m.get("functions", []):
        for blk in f.get("blocks", []):
            out = []
            for inst in blk.get("instructions", []):
                si = inst.get("sync_info") or {}
                waits = si.get("on_wait") or []
                keep = 0 if inst.get("opcode") == "Matmult" else 1
                if len(waits) > keep:
                    for wsub in waits[keep:]:
                        counter += 1
                        out.append({
                            "debug": inst.get("debug", 0),
                            "engine": inst["engine"],
                            "ins": [],
                            "outs": [],
                            "name": f"waitfix-{counter}",
                            "opcode": "NoOp",
                            "sync_info": {"on_update": [], "on_wait": [wsub]},
                        })
                    si["on_wait"] = waits[:keep]
                out.append(inst)
            blk["instructions"] = out
    return json.dumps(m).encode()


def _install_waitfix(nc):
    orig = nc.to_json_bytes

    def patched(*a, **k):
        return _fix_waits_json(orig(*a, **k))

    nc.to_json_bytes = patched


# ---------------------------------------------------------------- device code
def _build_module():
    nc = bass.Bass()

    hsT = nc.declare_dram_parameter("hsT", [128, 2, S], BF16, isOutput=False)
    wpack = nc.declare_dram_parameter("wpack", [128, 2, 224], BF16, isOutput=False)
    cspack = nc.declare_dram_parameter("cspack", [96, NQB, 2 * QB], BF16, isOutput=False)
    gtpack = nc.declare_dram_parameter("gtpack", [97, 2, 128], BF16, isOutput=False)
    mpack = nc.declare_dram_parameter("mpack", [128, 384], BF16, isOutput=False)
    out_part = nc.declare_dram_parameter("out_part", [128, 2, S], F32, isOutput=True)

    with tile.TileContext(nc) as tc:
        with (
            tc.tile_pool(name="const", bufs=1) as const,
            tc.tile_pool(name="qtp", bufs=4) as qtp,
            tc.tile_pool(name="qkp", bufs=2) as qkp,
            tc.tile_pool(name="ptp", bufs=8) as ptp,
            tc.tile_pool(name="up", bufs=2) as up,
            tc.tile_pool(name="outp", bufs=2) as outp,
            tc.tile_pool(name="ps_sc", bufs=2, space="PSUM") as ps_sc,
            tc.tile_pool(name="ps_pv", bufs=1, space="PSUM") as ps_pv,
            tc.tile_pool(name="ps_mm", bufs=3, space="PSUM") as ps_mm,
        ):
            # ---- persistent tiles
            hsT_sb = const.tile([128, 2, S], BF16)
            kT_rep = const.tile([128, S], BF16)
            # per chunk: col 0 = ones (softmax denominator row), cols 1-32 =
            # v, cols 33-63 = zeros (pad so PV writes a full 64-row band)
            v_all = const.tile([128, S // KC, 64], BF16)
            cs_sb = const.tile([96, NQB, 2 * QB], BF16)
            wpack_sb = const.tile([128, 2, 224], BF16)
            gt_sb = const.tile([97, 2, 128], BF16)
            mpack_sb = const.tile([128, 384], BF16)
            ones_t = const.tile([65, 64], BF16)
            eps_t = const.tile([65, 1], F32)
            dumm = const.tile([1, 8], BF16)

            tri_sb = mpack_sb[:, 0:128]
            repmat = mpack_sb[0:96, 256:384]

            # ---- PE warm-up while the first DMAs fly
            wtmp = const.tile([128, QB], BF16)
            nc.vector.memset(wtmp[:], 0.0)
            nc.vector.memset(ones_t[:], 1.0)
            nc.vector.memset(v_all[:, :, 0:1], 1.0)
            nc.vector.memset(v_all[:, :, 33:64], 0.0)
            nc.vector.memset(eps_t[:], 1e-6)
            nc.vector.memset(dumm[:], 0.0)
            for w in range(8):
                pwarm = ps_mm.tile([128, QB], F32, tag="mm", name="warm")
                nc.tensor.matmul(pwarm[:], wtmp[:, 0:128], wtmp[:],
                                 start=True, stop=True)
            # pre-trigger the ACT exp table load with a dummy op
            dume = const.tile([1, 8], BF16)
            nc.scalar.activation(out=dume[:], in_=dumm[:],
                                 func=mybir.ActivationFunctionType.Exp)

            # ---- DMAs, chunked per block, in the order blocks consume them
            nc.sync.dma_start(out=wpack_sb[:], in_=wpack[:])

            def dma_block_inputs(qb):
                sl = slice(QB * qb, QB * (qb + 1))
                nc.sync.dma_start(out=hsT_sb[:, :, sl], in_=hsT[:, :, sl])
                nc.sync.dma_start(out=cs_sb[:, qb, :], in_=cspack[:, qb, :])

            dma_block_inputs(0)
            nc.sync.dma_start(out=mpack_sb[:], in_=mpack[:])
            dma_block_inputs(1)
            nc.sync.dma_start(out=gt_sb[:], in_=gtpack[:])
            for qb in range(2, NQB):
                dma_block_inputs(qb)

            qt_state = {}

            live_pv = [None]

            def emit_prologue(qb):
                """q/k/v projections + RoPE + PE band replication for block qb.
                Zero-add fillers into the last actively-written PV accumulator
                keep the PE busy across the DVE-gated RoPE/cast chain."""
                def fill(n):
                    if live_pv[0] is None:
                        return
                    for _ in range(n):
                        nc.tensor.matmul(live_pv[0][0:64, :], wtmp[:, 0:64],
                                         wtmp[:], start=False, stop=False,
                                         skip_group_check=True)
                sl = slice(QB * qb, QB * (qb + 1))
                p_qk = ps_mm.tile([96, QB], F32, tag="mm", name="p_qk")
                p_qkr = ps_mm.tile([96, QB], F32, tag="mm", name="p_qkr")
                for c in range(2):
                    nc.tensor.matmul(p_qk[:], wpack_sb[:, c, 0:96],
                                     hsT_sb[:, c, sl],
                                     start=(c == 0), stop=(c == 1))
                for c in range(2):
                    nc.tensor.matmul(p_qkr[:], wpack_sb[:, c, 96:192],
                                     hsT_sb[:, c, sl],
                                     start=(c == 0), stop=(c == 1))
                t1 = qkp.tile([96, QB], BF16, tag="t1", name="t1")
                t2 = qkp.tile([96, QB], BF16, tag="t2", name="t2")
                qkT = qkp.tile([96, QB], BF16, tag="qkT", name="qkT")
                nc.vector.tensor_mul(t1[:], p_qk[:], cs_sb[:, qb, 0:QB])
                nc.vector.tensor_mul(t2[:], p_qkr[:], cs_sb[:, qb, QB:2 * QB])
                nc.vector.tensor_add(qkT[:], t1[:], t2[:])
                fill(2)

                # replicate q0/q1/k across the 4 row bands with the PE
                qt_h = [qtp.tile([128, QB], BF16, tag=f"qt{h}", name=f"qt{h}")
                        for h in range(2)]
                reps = []
                for a in range(3):
                    rp = ps_mm.tile([128, QB], F32, tag="mm", name=f"rep{a}")
                    nc.tensor.matmul(rp[:], repmat[32 * a:32 * (a + 1), :],
                                     qkT[32 * a:32 * (a + 1), :],
                                     start=True, stop=True,
                                     tile_position=(32 * a, 0),
                                     skip_group_check=True)
                    reps.append(rp)
                with nc.allow_low_precision(reason="bf16 q/k bands"):
                    nc.vector.tensor_copy(qt_h[0][:], reps[0][:])
                    nc.vector.tensor_copy(qt_h[1][:], reps[1][:])
                    nc.vector.tensor_copy(kT_rep[:, sl], reps[2][:])
                fill(2)

                # v projection for this block's 4 s-chunks
                vp = ps_mm.tile([128, 4, HD], F32, tag="mm", name="vproj")
                for s4 in range(4):
                    sblk = 4 * qb + s4
                    for c in range(2):
                        nc.tensor.matmul(
                            vp[:, s4, :],
                            hsT_sb[:, c, KC * sblk:KC * (sblk + 1)],
                            wpack_sb[:, c, 192:224],
                            start=(c == 0), stop=(c == 1),
                            skip_group_check=True)
                with nc.allow_low_precision(reason="bf16 v"):
                    nc.vector.tensor_copy(
                        v_all[:, 4 * qb:4 * qb + 4, 1:HD + 1], vp[:])
                qt_state[qb] = qt_h

            unit_idx = [0]

            def emit_scores_exp(qb, g, h, qt_h):
                """scores + exp for one (block, group, head); the causal mask
                on the two diagonal 128x128 sub-blocks is a DVE multiply by a
                lower-triangular table on the exp output."""
                diag0 = (g == 2 * qb)
                diag1 = (g == 2 * qb + 1)
                qlo = [0, 0]
                if diag0:
                    qlo = [0, KC]
                elif diag1:
                    qlo = [2 * KC, 3 * KC]
                sc = ps_sc.tile([128, 2 * QB], F32, tag="sc", name="sc")
                # for diag0 the exp is one contiguous op over [0:1024], so the
                # d=1 chunk writes from column QB (no uninitialized hole); for
                # diag1 the exp is split per chunk and both trims stay.
                qmm = [qlo[0], 0 if not diag1 else qlo[1]]
                for d in range(2):
                    ch = 2 * g + d
                    t = ch % 4
                    nc.tensor.matmul(
                        sc[:, QB * d + qmm[d]:QB * (d + 1)],
                        kT_rep[32 * t:32 * (t + 1), KC * ch:KC * (ch + 1)],
                        qt_h[h][32 * t:32 * (t + 1), qmm[d]:],
                        start=True, stop=True,
                        tile_position=(32 * t, 0),
                        skip_group_check=True)
                pt = ptp.tile([128, 2 * QB], BF16, tag="pt", name="pt")
                ui = unit_idx[0]
                unit_idx[0] += 1
                offl = (ui % OFFLOAD_MOD) < OFFLOAD_CNT
                regions = ([(qlo[0], 2 * QB)] if not diag1 else
                           [(qlo[0], QB), (QB + qlo[1], 2 * QB)])
                for (r0, r1) in regions:
                    if offl:
                        # 1-op DVE Schraudolph fast-exp: int16 bf16-bits
                        with nc.allow_low_precision(reason="schraudolph exp"):
                            nc.vector.tensor_scalar(
                                pt[:, r0:r1].bitcast(I16), sc[:, r0:r1],
                                SCH_MUL, SCH_ADD,
                                mybir.AluOpType.mult, mybir.AluOpType.add)
                    else:
                        nc.scalar.activation(
                            out=pt[:, r0:r1], in_=sc[:, r0:r1],
                            func=mybir.ActivationFunctionType.Exp,
                            scale=float(SCALE))
                if diag0 or diag1:
                    for d in range(2):
                        dg = 2 * g + d - 4 * qb
                        w0 = QB * d + KC * dg
                        nc.vector.tensor_mul(pt[:, w0:w0 + KC],
                                             pt[:, w0:w0 + KC], tri_sb)
                return pt

            def emit_pv(entry):
                """col-tiled PV pair: h0 -> pvacc[0:64], h1 -> pvacc[64:128]."""
                qb_, g_, pts, pvacc_ = entry
                first = (g_ == 0)
                last = (g_ == 2 * qb_ + 1)
                if first:
                    live_pv[0] = pvacc_
                if not first:
                    # zero-add fillers (N=256 quantum): K=128 so their row
                    # groups overlap everything; the PE runs them back-to-back
                    for _ in range(FILL_BY_BLOCK[qb_]):
                        nc.tensor.matmul(pvacc_[0:64, :], wtmp[:, 0:64],
                                         wtmp[:], start=False, stop=False,
                                         skip_group_check=True)
                for d in range(2):
                    ch = 2 * g_ + d
                    dg = ch - 4 * qb_
                    diag_ = (g_ >= 2 * qb_)
                    cols = KC * dg if diag_ else 0
                    for h in range(2):
                        # start marks pending-zero for this matmul's own
                        # partitions only, so both heads' first PV start=True
                        nc.tensor.matmul(
                            pvacc_[64 * h:64 * h + 64, cols:],
                            v_all[:, ch, :],
                            pts[h][:, QB * d + cols:QB * (d + 1)],
                            start=(first and d == 0),
                            stop=(last and d == 1),
                            tile_position=(0, 64 * h),
                            skip_group_check=True)

            norm_state = {}

            def emit_norm_sq(qb):
                """stage 1 of the softmax reciprocal 1/d = exp(-0.5*ln(d^2)):
                the squaring keeps the ln input positive on the (unused) rows
                holding unnormalized outputs, so no NaNs; +1e-6 keeps the
                zero-pad rows' ln finite. Must be emitted after block qb's
                last PV and before block qb+1's first PV (pvacc is
                single-buffered), as must the u cast."""
                pvacc = pv_state.pop(qb)
                sq = up.tile([65, QB], F32, tag="sq", name="sq")
                nc.scalar.activation(out=sq[:], in_=pvacc[0:65, :],
                                     func=mybir.ActivationFunctionType.Square,
                                     bias=eps_t[:])
                u_bf = up.tile([97, QB], BF16, tag="u", name="u")
                with nc.allow_low_precision(reason="unnorm attn bf16"):
                    nc.vector.tensor_copy(u_bf[:], pvacc[0:97, :])
                norm_state[qb] = (u_bf, sq)

            def emit_norm_ln(qb):
                u_bf, sq = norm_state[qb]
                lg = up.tile([65, QB], F32, tag="lg", name="lg")
                nc.scalar.activation(out=lg[:], in_=sq[:],
                                     func=mybir.ActivationFunctionType.Ln)
                norm_state[qb] = (u_bf, lg)

            def emit_norm_rc(qb):
                u_bf, lg = norm_state[qb]
                rc_bf = up.tile([65, QB], BF16, tag="rcb", name="rcb")
                with nc.allow_low_precision(reason="bf16 softmax denom"):
                    nc.scalar.activation(out=rc_bf[:], in_=lg[:],
                                         func=mybir.ActivationFunctionType.Exp,
                                         scale=-0.5)
                norm_state[qb] = (u_bf, rc_bf)

            def fill_live(n):
                if live_pv[0] is None:
                    return
                for _ in range(n):
                    nc.tensor.matmul(live_pv[0][0:64, :], wtmp[:, 0:64],
                                     wtmp[:], start=False, stop=False,
                                     skip_group_check=True)

            def emit_norm_mid(qb):
                """PE broadcast of the reciprocal across the 33 output rows."""
                u_bf, rc_bf = norm_state[qb]
                fill_live(2)
                bc_ps = ps_mm.tile([128, QB], F32, tag="mm", name="bc")
                for h in range(2):
                    nc.tensor.matmul(bc_ps[64 * h:64 * h + 64, :],
                                     ones_t[64 * h:64 * h + 1, :],
                                     rc_bf[64 * h:64 * h + 1, :],
                                     start=True, stop=True,
                                     tile_position=(64 * h, 64 * h),
                                     skip_group_check=True)
                norm_state[qb] = (u_bf, rc_bf, bc_ps)

            def emit_norm_post(qb):
                """normalize + output-project + store block qb."""
                u_bf, rc_bf, bc_ps = norm_state.pop(qb)
                fill_live(2)
                slp = slice(QB * qb, QB * (qb + 1))
                nT = up.tile([97, QB], BF16, tag="nT", name="nT")
                with nc.allow_low_precision(reason="softmax norm bf16"):
                    nc.vector.tensor_mul(nT[:], u_bf[:], bc_ps[0:97, :])
                po_sb = outp.tile([128, 2, QB], F32, tag="po", name="po")
                for m in range(2):
                    po = ps_mm.tile([128, QB], F32, tag="mm", name="outproj")
                    # K=65 slice (rows 33-64 are zeros) makes the two MMs'
                    # row groups overlap at strip 2, so the HW serializes
                    # them -- concurrent row-tiled MMs accumulating into the
                    # same PSUM elements race and fault.
                    nc.tensor.matmul(po[:], gt_sb[0:65, m, :], nT[0:65, :],
                                     start=True, stop=False,
                                     skip_group_check=True)
                    nc.tensor.matmul(po[:], gt_sb[64:97, m, :], nT[64:97, :],
                                     start=False, stop=True,
                                     tile_position=(64, 0),
                                     skip_group_check=True)
                    nc.vector.tensor_copy(po_sb[:, m, :], po[:])
                    for hh in range(2):
                        cs = slice(QB * qb + 256 * hh, QB * qb + 256 * (hh + 1))
                        nc.sync.dma_start(out=out_part[:, m, cs],
                                          in_=po_sb[:, m, 256 * hh:256 * (hh + 1)])

            # ---- main pipeline: ascending blocks.
            # PV runs 2 groups behind its scores/exp (defer queue), crossing
            # block boundaries: at block qb, the g=0 and g=1 pops emit the
            # previous block's last two PV pairs, so norm_pre(qb-1) goes right
            # after the g=1 pop and the g=2 pop emits this block's first PV.
            emit_prologue(0)
            emit_prologue(1)
            emit_prologue(2)
            pv_state = {}
            defer = []
            next_plg = [3]
            for qb in range(NQB):
                qt_h = qt_state.pop(qb)
                pvacc = ps_pv.tile([128, QB], F32, tag="pvacc", name="pvacc")
                pv_state[qb] = pvacc
                ngroups = 2 * (qb + 1)
                for g in range(ngroups):
                    pt0 = emit_scores_exp(qb, g, 0, qt_h)
                    pt1 = emit_scores_exp(qb, g, 1, qt_h)
                    defer.append((qb, g, [pt0, pt1], pvacc))
                    if len(defer) > 2:
                        emit_pv(defer.pop(0))
                    if qb > 0:
                        if ngroups >= 6:
                            if g == 1:
                                emit_norm_sq(qb - 1)
                            elif g == 2:
                                emit_norm_ln(qb - 1)
                            elif g == 3:
                                emit_norm_rc(qb - 1)
                            elif g == 4:
                                emit_norm_mid(qb - 1)
                            elif g == 5:
                                emit_norm_post(qb - 1)
                        else:
                            if g == 1:
                                emit_norm_sq(qb - 1)
                                emit_norm_ln(qb - 1)
                            elif g == 2:
                                emit_norm_rc(qb - 1)
                                emit_norm_mid(qb - 1)
                            elif g == 3:
                                emit_norm_post(qb - 1)
                    if next_plg[0] < NQB and next_plg[0] - qb <= 3 \
                            and g == min(1 if qb < 3 else 4, ngroups - 1):
                        emit_prologue(next_plg[0])
                        next_plg[0] += 1

            # ---- drain + final norm
            for entry in defer:
                emit_pv(entry)
            emit_norm_sq(NQB - 1)
            emit_norm_ln(NQB - 1)
            emit_norm_rc(NQB - 1)
            emit_norm_mid(NQB - 1)
            emit_norm_post(NQB - 1)

    _install_waitfix(nc)
    return nc


_NC_CACHE = {}


def _get_nc():
    if "nc" not in _NC_CACHE:
        _NC_CACHE["nc"] = _build_module()
    return _NC_CACHE["nc"]


# ---------------------------------------------------------------- host side
def _rope_tables():
    inv = 1.0 / (ROPE_BASE ** (np.arange(0, HD, 2, dtype=np.float64) / HD))
    t = np.arange(S, dtype=np.float64)
    freqs = np.outer(t, inv)                     # [S, 16]
    emb = np.concatenate([freqs, freqs], axis=1)  # [S, 32]
    cosT = np.cos(emb).T.astype(np.float32)      # [32, S]
    sinT = np.sin(emb).T.astype(np.float32)
    return np.tile(cosT, (3, 1)), np.tile(sinT, (3, 1))   # [96, S]


def _rot_rows(w):
    # rows of (rotate_half o) projection: row d<16 -> -w[d+16]; d>=16 -> w[d-16]
    return np.concatenate([-w[16:32], w[0:16]], axis=0)


def kernel(hidden_states, Wq, Wk, Wv, Wo):
    hidden_states = np.asarray(hidden_states, dtype=np.float32)
    Wq = np.asarray(Wq, dtype=np.float32)
    Wk = np.asarray(Wk, dtype=np.float32)
    Wv = np.asarray(Wv, dtype=np.float32)
    Wo = np.asarray(Wo, dtype=np.float32)

    cosT, sinT = _rope_tables()   # [96, S] each
    cspack = np.empty((96, NQB, 2 * QB), np.float32)
    for qb in range(NQB):
        sl = slice(QB * qb, QB * (qb + 1))
        cspack[:, qb, 0:QB] = cosT[:, sl]
        cspack[:, qb, QB:] = sinT[:, sl]

    # mpack: [tri (lower-triangular causal keep-mask) | unused | repmat3]
    mpack = np.zeros((128, 384), np.float32)
    mpack[:, 0:128] = (np.arange(KC)[:, None] <= np.arange(KC)[None, :])
    for a in range(3):
        for t in range(4):
            mpack[32 * a:32 * (a + 1), 256 + 32 * t:256 + 32 * (t + 1)] += \
                np.eye(32, dtype=np.float32)

    hsT_b = []
    for b in range(B):
        ht = np.ascontiguousarray(hidden_states[b].T)   # [256, S]
        hsT_b.append(np.ascontiguousarray(
            ht.reshape(2, 128, S).transpose(1, 0, 2)).astype(NP_BF16))

    in_maps = []
    for core in range(8):
        b, j = core // 4, core % 4
        Wq_h = Wq[64 * j:64 * j + 64]            # [64, 256]
        Wk_j = Wk[32 * j:32 * j + 32]            # [32, 256]
        Wqk = np.concatenate([Wq_h, Wk_j], axis=0)           # [96, 256]
        Wqkrot = np.concatenate(
            [_rot_rows(Wq_h[0:32]), _rot_rows(Wq_h[32:64]), _rot_rows(Wk_j)],
            axis=0)                                          # [96, 256]
        Wv_j = Wv[32 * j:32 * j + 32]            # [32, 256]
        # wpack[p, c, :] = [Wqk[:,128c+p] | Wqkrot[:,128c+p] | Wv[:,128c+p]]
        wpack = np.empty((128, 2, 224), np.float32)
        for c in range(2):
            cols = slice(128 * c, 128 * (c + 1))
            wpack[:, c, 0:96] = Wqk[:, cols].T
            wpack[:, c, 96:192] = Wqkrot[:, cols].T
            wpack[:, c, 192:224] = Wv_j[:, cols].T

        G = Wo[:, 64 * j:64 * j + 64]            # [256, 64]
        gtpack = np.zeros((97, 2, 128), np.float32)
        for m in range(2):
            rows = slice(128 * m, 128 * (m + 1))
            gtpack[1:33, m, :] = G[rows, 0:32].T
            gtpack[65:97, m, :] = G[rows, 32:64].T

        in_maps.append({
            "hsT": hsT_b[b],
            "wpack": wpack.astype(NP_BF16),
            "cspack": cspack.astype(NP_BF16),
            "gtpack": gtpack.astype(NP_BF16),
            "mpack": mpack.astype(NP_BF16),
        })

    nc = _get_nc()
    res = run_bass_kernel_spmd(nc, in_maps, list(range(8)), trace=False)

    out = np.empty((B, S, H), np.float32)
    for b in range(B):
        acc = np.zeros((H, S), np.float32)
        for j in range(4):
            r = np.asarray(res.results[4 * b + j]["out_part"], dtype=np.float32)
            acc += r.transpose(1, 0, 2).reshape(H, S)
        out[b] = acc.T
    return out
